# revision 27
# baseline (speedup 1.0000x reference)
"""Trainium2 Bass kernel for CLSAggregator: 6-layer dense transformer encoder
(ALiBi attention + SwiGLU MLP), B=4, S=1024, D=768, H=16, FF=3072.

Sharding: tokens split (batch, seq-half) -> 8 cores, 512 tokens each.
Per layer each core computes LN1/QKV/attention/Wo/LN2/SwiGLU for its token
slab. K,V are combined within core pairs via ReduceScatter(add) over a
duplicated input; each core recovers the partner's K,V as (sum - own),
which keeps the program SPMD-uniform. Attention runs in two phases:
the diagonal phase (own keys, locally staged, ALiBi added via a diagonal
matmul against a |i-j| table) runs before the collective result is
needed; the partner phase gets ALiBi for free through two augmented
contraction rows (position features scaled by the per-head slope, exact
in fp16 via /64 scaling) folded into the scores matmul.

Self-contained: hardcodes all shapes; host side folds LN weights into the
projection weights and precomputes ALiBi tables/augmented features.
"""
import math
import os

import numpy as np
import ml_dtypes

import concourse.bass as bass
import concourse.mybir as mybir
import concourse.tile as tile
from concourse import bacc
from concourse.bass_utils import run_bass_kernel_spmd
from concourse.masks import make_identity

F32 = mybir.dt.float32
F32R = mybir.dt.float32r
BF16 = mybir.dt.bfloat16
FP16 = mybir.dt.float16
AF = mybir.ActivationFunctionType
OP = mybir.AluOpType

L, H, D, FF = 6, 16, 768, 3072
B, NSEQ = 4, 1023
S = NSEQ + 1            # 1024
HD = D // H             # 48
EPS = 1e-5
NCORES = 8
T = S // 2              # 512 tokens per core
KT = D // 128           # 6 feature k-tiles
TT = T // 128           # 4 token tiles
FT = FF // 128          # 24 ff tiles
QK_PAD = H * 64         # 1024 padded q (or k) rows
QK_D = H * 48           # 768 dense q (or k) rows
VW = H * 49             # 784 v cols incl per-head ones-aug column
KELEM = QK_D * T        # K staged dense
VELEM = T * VW
KVLEN = KELEM + VELEM

_DTMAP = {"bf16": BF16, "f32r": F32R, "fp16": FP16}
DT_A = _DTMAP[os.environ.get("KDT_A", "fp16")]   # attention operands / Wo / Wd / KV
DT_H = _DTMAP[os.environ.get("KDT_H", "fp16")]   # QKV & gate/up weights + acts
FP8 = mybir.dt.float8e4
FP8_FFN = os.environ.get("KFP8", "0") == "1"     # fp8 DoubleRow FFN GEMMs
# fp8 pre-scales, chosen so every fp8 tensor stays well inside e4m3's
# +-448 range: gate weights x64 (silu's input scale divides it back out),
# up weights x16 (rides into h3: |h3| ~ 16*0.3sigma ~ 5, max ~150),
# down weights x64 (PSUM f32 holds 1024x; divided out at eviction).
SCALE_G = 64.0
SCALE_U = 16.0
SCALE_D = 64.0

_NC_CACHE = {}


def head_spans(h):
    """Destination spans of head h's 48 rows inside 128-row feature tiles:
    list of (tile, dst_row, src_row_within_head, length)."""
    r = 48 * h
    g0, r0 = r // 128, r % 128
    if r0 + 48 <= 128:
        return [(g0, r0, 0, 48)]
    c = 128 - r0
    return [(g0, r0, 0, c), (g0 + 1, 0, c, 48 - c)]


def build_nc(use_bqk, use_bgu, l_run=L, bare=False, slopes=None, nrep=1):
    nc = bacc.Bacc("TRN2", target_bir_lowering=False, debug=False,
                   enable_asserts=True, num_devices=NCORES)

    # ALiBi band sparsity: for a (head-pair, k-tile, q-block) whose minimum
    # |i-j| (over BOTH core halves, so the program stays SPMD-uniform) makes
    # slope*dist >= CUT, every P in the block is < e^2-CUT of the max --
    # negligible vs the softmax denominator -- so its scores/exp/AV work is
    # skipped. Gated by the pair's smaller slope (head 2j+1).
    if slopes is None:
        ratio = 2.0 ** (-8.0 / H)
        slopes = np.array([ratio ** (i + 1) for i in range(H)], np.float64)
    CUT = 15.0
    keep_runs = {}
    for j in range(8):
        sl = float(slopes[2 * j + 1])
        for ph in (0, 1):
            for dd in range(4):
                tts = [tt for tt in range(4)
                       if sl * ((0 if dd == tt else (abs(dd - tt) - 1) * 128 + 1)
                                if ph == 0
                                else (4 - abs(dd - tt)) * 128 - 127) < CUT]
                rs = []
                for tt in tts:
                    if rs and rs[-1][1] == tt * 128:
                        rs[-1][1] = (tt + 1) * 128
                    else:
                        rs.append([tt * 128, (tt + 1) * 128])
                keep_runs[(j, ph, dd)] = [(a, b - a) for a, b in rs]

    # ---- I/O ----
    x0_d = nc.dram_tensor("x0", [T, D], F32, kind="ExternalInput")
    dist_d = nc.dram_tensor("dist", [T, T], FP16, kind="ExternalInput")
    sid_d = nc.dram_tensor("sid", [128, H * 128], FP16, kind="ExternalInput")
    kaug_d = nc.dram_tensor("kaug", [2 * H, T], FP16, kind="ExternalInput")
    qaug_d = nc.dram_tensor("qaug", [2 * H, T], FP16, kind="ExternalInput")
    wqk_d = nc.dram_tensor("wqk", [L, D, 2 * QK_D], DT_H, kind="ExternalInput")
    wv_d = nc.dram_tensor("wv", [L, D, VW], DT_H, kind="ExternalInput")
    bv_d = nc.dram_tensor("bv", [L, 1, VW], DT_H, kind="ExternalInput")
    wo_d = nc.dram_tensor("wo", [L, D, D], DT_A, kind="ExternalInput")
    ffn8 = FP8_FFN and not use_bgu
    wgu_d = nc.dram_tensor("wgu", [L, D, 2 * FF], FP8 if ffn8 else DT_H,
                           kind="ExternalInput")
    wd_d = nc.dram_tensor("wd", [L, FF, D], FP8 if ffn8 else DT_A,
                          kind="ExternalInput")
    if use_bqk:
        bqk_d = nc.dram_tensor("bqk", [L, 1, 2 * QK_D], DT_H, kind="ExternalInput")
    if use_bgu:
        bg_d = nc.dram_tensor("bg", [L, 1, FF], DT_H, kind="ExternalInput")
        bu_d = nc.dram_tensor("bu", [L, 1, FF], DT_H, kind="ExternalInput")
    finw_d = nc.dram_tensor("finw", [1, D], F32, kind="ExternalInput")
    finb_d = nc.dram_tensor("finb", [1, D], F32, kind="ExternalInput")
    y_d = nc.dram_tensor("y", [1, D], F32, kind="ExternalOutput")

    if bare:
        # overhead-measurement baseline: zero compute, same I/O signature
        with tile.TileContext(nc) as tc:
            with tc.tile_pool(name="pb", bufs=1) as pb:
                yt = pb.tile([1, D], F32, tag="fy", name="fy")
                nc.vector.memset(yt[:], 0.0)
                nc.sync.dma_start(y_d.ap(), yt[:])
        nc.compile()
        return nc

    with tile.TileContext(nc) as tc:
        with (
            tc.tile_pool(name="p1", bufs=1) as p1,
            tc.tile_pool(name="p2", bufs=2) as p2,
            tc.tile_pool(name="p3", bufs=3) as p3,
            tc.tile_pool(name="p4", bufs=4) as p4,
            tc.tile_pool(name="psmm", bufs=2, space="PSUM") as psmm,
            tc.tile_pool(name="dram", bufs=2, space="DRAM") as dram,
        ):
            # ---- persistent tiles ----
            ident = p1.tile([128, 128], DT_A, tag="ident", name="ident")
            make_identity(nc, ident[:])
            ones_f = p1.tile([1, 128], F32, tag="ones_f", name="ones_f")
            nc.vector.memset(ones_f[:], 1.0)
            ones_h = p1.tile([1, 128], DT_H, tag="ones_h", name="ones_h")    # K=1 lhsT for v bias
            nc.vector.tensor_copy(ones_h[:], ones_f[0:1, 0:128])
            if use_bqk or use_bgu:
                ones_row = p1.tile([1, T], DT_H, tag="ones_row", name="ones_row")
                for c4 in range(4):
                    nc.scalar.copy(ones_row[0:1, c4 * 128:(c4 + 1) * 128], ones_f[:])
            epst = p1.tile([128, 1], F32, tag="epst", name="epst")
            nc.vector.memset(epst[:], EPS)
            maskf = p1.tile([1, 64], F32, tag="maskf", name="maskf")
            nc.vector.memset(maskf[:], 0.0)
            nc.vector.memset(maskf[0:1, 0:48], 1.0)
            mask48 = p1.tile([1, 64], DT_A, tag="mask48", name="mask48")
            nc.vector.tensor_copy(mask48[:], maskf[:])

            # Pre-zero all PSUM banks so never-written pad regions read as
            # finite values.
            for zi in range(2):
                z = psmm.tile([128, 512], F32, tag="mm", name="mm")
                nc.vector.memset(z[:], 0.0)
            for zi in range(3):
                z = psmm.tile([128, 1024], F32, tag="mmp", name="mmp", bufs=3)
                nc.vector.memset(z[:], 0.0)

            distT = [p1.tile([128, T], FP16, tag=f"dist{dd}", name=f"dist{dd}")
                     for dd in range(4)]
            for dd in range(4):
                nc.sync.dma_start(distT[dd][:], dist_d.ap()[dd * 128:(dd + 1) * 128, :])
            sid = p1.tile([128, H * 128], FP16, tag="sid", name="sid")
            nc.sync.dma_start(sid[:], sid_d.ap())
            kaug_sb = p1.tile([2 * H, T], FP16, tag="kaug", name="kaug")
            nc.sync.dma_start(kaug_sb[:], kaug_d.ap())
            qaug_sb = p1.tile([2 * H, T], FP16, tag="qaug", name="qaug")
            nc.sync.dma_start(qaug_sb[:], qaug_d.ap())

            x = [p1.tile([128, D], F32, tag=f"x{t}", name=f"x{t}") for t in range(TT)]

            # persistent padded-head K/Q tiles; pad rows are zeroed once and
            # never written afterwards (per-layer DMAs touch head spans only)
            kst = [p1.tile([128, T], DT_A, tag=f"kst{m}", name=f"kst{m}")
                   for m in range(8)]
            qT = [p1.tile([128, T], DT_A, tag=f"qT{m}", name=f"qT{m}")
                  for m in range(8)]
            kpart = [p1.tile([128, T], DT_A, tag=f"kpart{m}", name=f"kpart{m}")
                     for m in range(8)]
            for m in range(8):
                nc.vector.memset(kst[m][:], 0.0)
                nc.vector.memset(qT[m][:], 0.0)
                nc.vector.memset(kpart[m][:], 0.0)

            def layernorm_to(src_tiles, dst_tiles):
                """LN over features (free dim of token-major src); transposed
                feature-major DT_H output into dst_tiles (KT x [128, T])."""
                for t in range(TT):
                    st = p2.tile([128, 12], F32, tag="bnst", name="bnst")
                    nc.vector.bn_stats(st[:, 0:6], src_tiles[t][:, 0:384])
                    nc.vector.bn_stats(st[:, 6:12], src_tiles[t][:, 384:768])
                    ag = p2.tile([128, 2], F32, tag="bnag", name="bnag")
                    nc.vector.bn_aggr(ag[:], st[:])
                    nmean = p2.tile([128, 1], F32, tag="nmean", name="nmean")
                    nc.scalar.mul(nmean[:], ag[:, 0:1], -1.0)
                    stdt = p2.tile([128, 1], F32, tag="stdt", name="stdt")
                    nc.scalar.activation(stdt[:], ag[:, 1:2], AF.Sqrt, bias=epst[:])
                    rstd = p2.tile([128, 1], F32, tag="rstd", name="rstd")
                    nc.vector.reciprocal(rstd[:], stdt[:])
                    hn = p2.tile([128, D], DT_H, tag="hnorm", name="hnorm")
                    nc.vector.tensor_scalar(hn[:], src_tiles[t][:], nmean[:], rstd[:],
                                            OP.add, OP.mult)
                    for d in range(KT):
                        pst = psmm.tile([128, 128], DT_H, tag="mm", name="mm")
                        nc.tensor.transpose(pst[:], hn[:, d * 128:(d + 1) * 128],
                                            ident[:])
                        nc.vector.tensor_copy(dst_tiles[d][:, t * 128:(t + 1) * 128],
                                              pst[:])

            def attn_phase(j, ktiles, qtiles, vtiles, diag, psav):
                for dd in range(4):
                    runs = keep_runs[(j, 0 if diag else 1, dd)]
                    if not runs:
                        continue
                    pss = psmm.tile([128, 2, T], F32, tag="mmp", name="mmp", bufs=3)
                    pt = p4.tile([128, 2, T], DT_A, tag="p", name="p", bufs=3)
                    for hi, hh in ((0, 2 * j), (1, 2 * j + 1)):
                        ft, qb = hh // 2, (hh % 2) * 64
                        for (c0, cn) in runs:
                            nc.tensor.matmul(
                                pss[:, hi, c0:c0 + cn],
                                ktiles[ft][qb:qb + 64, dd * 128:(dd + 1) * 128],
                                qtiles[ft][qb:qb + 64, c0:c0 + cn],
                                start=True, stop=not diag)
                            if diag:
                                nc.tensor.matmul(
                                    pss[:, hi, c0:c0 + cn],
                                    sid[:, hh * 128:(hh + 1) * 128],
                                    distT[dd][:, c0:c0 + cn],
                                    start=False, stop=True)
                    for (c0, cn) in runs:
                        nc.scalar.activation(pt[:, :, c0:c0 + cn],
                                             pss[:, :, c0:c0 + cn], AF.Exp)
                    for hi, avb in ((0, 0), (1, 64)):
                        hh = 2 * j + hi
                        for (c0, cn) in runs:
                            nc.tensor.matmul(
                                psav[avb:avb + 64, c0:c0 + cn],
                                vtiles[dd][:, 49 * hh:49 * hh + 64],
                                pt[:, hi, c0:c0 + cn],
                                start=False, stop=True,
                                tile_position=(0, avb), skip_group_check=True)

            for rep in range(nrep):
                for t in range(TT):
                    nc.sync.dma_start(x[t][:], x0_d.ap()[t * 128:(t + 1) * 128, :])

                for l_i in range(l_run):
                    l = l_i % L
                    # ================= attention =================
                    # early weight preloads for this layer
                    wv_sb = p2.tile([128, KT, VW], DT_H, tag="wv_sb", name="wv_sb",
                                    bufs=1)
                    nc.sync.dma_start(
                        wv_sb[:], wv_d.ap()[l].rearrange("(o p) n -> p o n", p=128))
                    wo_sb = p2.tile([128, KT, D], DT_A, tag="wo_sb", name="wo_sb",
                                    bufs=1)
                    nc.sync.dma_start(
                        wo_sb[:], wo_d.ap()[l].rearrange("(o p) n -> p o n", p=128))

                    hT = [p1.tile([128, T], DT_H, tag=f"hT{k}", name=f"hT{k}")
                          for k in range(KT)]
                    layernorm_to(x, hT)

                    kv_in2 = dram.tile([2 * KVLEN], DT_A, tag="kv_in2", name="kv_in2")
                    kv_sum = dram.tile([KVLEN], DT_A, tag="kv_sum", name="kv_sum")

                    def kv_in2_k(c):
                        return kv_in2[c * KVLEN:c * KVLEN + KELEM].rearrange(
                            "(r c) -> r c", c=T)

                    def kv_in2_v(c):
                        return kv_in2[c * KVLEN + KELEM:(c + 1) * KVLEN].rearrange(
                            "(r c) -> r c", c=VW)

                    if use_bqk:
                        bqkt = p2.tile([1, 2 * QK_PAD], DT_H, tag="bqk_s", name="bqk_s")
                        nc.sync.dma_start(bqkt[:], bqk_d.ap()[l])

                    # K projection (dense feature-major); staged dense, then
                    # expanded into the padded per-head layout for scores
                    kdense = []
                    for mp in range(3):
                        wt = p2.tile([128, KT, 256], DT_H, tag="wqk_s", name="wqk_s")
                        nc.sync.dma_start(
                            wt[:], wqk_d.ap()[l][:, QK_D + mp * 256:QK_D + (mp + 1) * 256]
                            .rearrange("(o p) n -> p o n", p=128))
                        for mm_ in range(2):
                            m = 2 * mp + mm_
                            ps = psmm.tile([128, T], F32, tag="mm", name="mm")
                            for k in range(KT):
                                nc.tensor.matmul(ps[:], wt[:, k, mm_ * 128:(mm_ + 1) * 128],
                                                 hT[k][:], start=(k == 0),
                                                 stop=(k == KT - 1 and not use_bqk))
                            if use_bqk:
                                nc.tensor.matmul(
                                    ps[:], bqkt[0:1, QK_D + m * 128:QK_D + (m + 1) * 128],
                                    ones_row[:], start=False, stop=True)
                            kt_t = p1.tile([128, T], DT_A, tag=f"kdn{m}", name=f"kdn{m}")
                            nc.vector.tensor_copy(kt_t[:], ps[:])
                            nc.sync.dma_start(kv_in2_k(0)[m * 128:(m + 1) * 128, :], kt_t[:])
                            nc.sync.dma_start(kv_in2_k(1)[m * 128:(m + 1) * 128, :], kt_t[:])
                            kdense.append(kt_t)
                    for h in range(H):
                        ft, qb = h // 2, (h % 2) * 64
                        for (g, r0, sr, ln) in head_spans(h):
                            nc.sync.dma_start(kst[ft][qb + sr:qb + sr + ln, :],
                                              kdense[g][r0:r0 + ln, :])

                    # V projection (token-major with ones-aug cols); local + staged
                    bvt = p1.tile([1, VW], DT_H, tag="bv_s", name="bv_s")
                    nc.sync.dma_start(bvt[:], bv_d.ap()[l])
                    vloc = []
                    for t in range(TT):
                        psv = psmm.tile([128, 1024], F32, tag="mmp", name="mmp", bufs=3)
                        vst = p1.tile([128, VW + 16], DT_A, tag=f"vloc{t}", name=f"vloc{t}")
                        nc.vector.memset(vst[:, VW:VW + 16], 0.0)
                        for (n0, nlen) in ((0, 512), (512, VW - 512)):
                            for k in range(KT):
                                nc.tensor.matmul(psv[:, n0:n0 + nlen],
                                                 hT[k][:, t * 128:(t + 1) * 128],
                                                 wv_sb[:, k, n0:n0 + nlen],
                                                 start=(k == 0), stop=False)
                            nc.tensor.matmul(psv[:, n0:n0 + nlen], ones_h[:],
                                             bvt[0:1, n0:n0 + nlen], start=False, stop=True)
                            nc.vector.tensor_copy(vst[:, n0:n0 + nlen], psv[:, n0:n0 + nlen])
                        nc.sync.dma_start(kv_in2_v(0)[t * 128:(t + 1) * 128, :],
                                          vst[:, 0:VW])
                        nc.sync.dma_start(kv_in2_v(1)[t * 128:(t + 1) * 128, :],
                                          vst[:, 0:VW])
                        vloc.append(vst)

                    # pair-sum of K,V: each core later recovers partner = sum - own
                    nc.gpsimd.collective_compute(
                        "ReduceScatter", OP.add,
                        replica_groups=[[0, 1], [2, 3], [4, 5], [6, 7]],
                        ins=[kv_in2[:].opt()],
                        outs=[kv_sum[:].opt()],
                    )

                    # Q projection (dense) -> expand to padded layout + aug rows
                    qdense = []
                    for mp in range(3):
                        wt = p2.tile([128, KT, 256], DT_H, tag="wqk_s", name="wqk_s")
                        nc.sync.dma_start(
                            wt[:], wqk_d.ap()[l][:, mp * 256:(mp + 1) * 256]
                            .rearrange("(o p) n -> p o n", p=128))
                        for mm_ in range(2):
                            m = 2 * mp + mm_
                            ps = psmm.tile([128, T], F32, tag="mm", name="mm")
                            for k in range(KT):
                                nc.tensor.matmul(ps[:], wt[:, k, mm_ * 128:(mm_ + 1) * 128],
                                                 hT[k][:], start=(k == 0),
                                                 stop=(k == KT - 1 and not use_bqk))
                            if use_bqk:
                                nc.tensor.matmul(ps[:], bqkt[0:1, m * 128:(m + 1) * 128],
                                                 ones_row[:], start=False, stop=True)
                            qt = p1.tile([128, T], DT_A, tag=f"qdn{m}", name=f"qdn{m}")
                            nc.vector.tensor_copy(qt[:], ps[:])
                            qdense.append(qt)
                    for h in range(H):
                        ft, qb = h // 2, (h % 2) * 64
                        for (g, r0, sr, ln) in head_spans(h):
                            nc.sync.dma_start(qT[ft][qb + sr:qb + sr + ln, :],
                                              qdense[g][r0:r0 + ln, :])
                    for hh in range(H):
                        ft, qb = hh // 2, (hh % 2) * 64
                        nc.sync.dma_start(qT[ft][qb + 48:qb + 50, :],
                                          qaug_sb[2 * hh:2 * hh + 2, :])

                    # ---- diagonal phase: own keys, ALiBi via sid @ dist ----
                    o_diag = []
                    for j in range(8):
                        psav = psmm.tile([128, T], F32, tag="mm", name="mm")
                        nc.vector.memset(psav[:], 0.0)
                        attn_phase(j, kst, qT, vloc, True, psav)
                        od = p1.tile([128, T], DT_A, tag=f"odiag{j}", name=f"odiag{j}")
                        nc.vector.tensor_copy(od[:], psav[:])
                        o_diag.append(od)

                    # ---- partner K/V = sum - own; insert aug rows into K ----
                    kv_sum_k = kv_sum[0:KELEM].rearrange("(r c) -> r c", c=T)
                    kv_sum_v = kv_sum[KELEM:KVLEN].rearrange("(r c) -> r c", c=VW)
                    kpden = []
                    for m in range(6):
                        ksum_t = p2.tile([128, T], DT_A, tag="ksum", name="ksum")
                        nc.sync.dma_start(ksum_t[:], kv_sum_k[m * 128:(m + 1) * 128, :])
                        kp = p1.tile([128, T], DT_A, tag=f"kpd{m}", name=f"kpd{m}")
                        nc.vector.tensor_tensor(kp[:], ksum_t[:], kdense[m][:],
                                                OP.subtract)
                        kpden.append(kp)
                    for h in range(H):
                        ft, qb = h // 2, (h % 2) * 64
                        for (g, r0, sr, ln) in head_spans(h):
                            nc.sync.dma_start(kpart[ft][qb + sr:qb + sr + ln, :],
                                              kpden[g][r0:r0 + ln, :])
                    for hh in range(H):
                        ft, qb = hh // 2, (hh % 2) * 64
                        nc.sync.dma_start(kpart[ft][qb + 48:qb + 50, :],
                                          kaug_sb[2 * hh:2 * hh + 2, :])
                    vpart = []
                    for t in range(TT):
                        vsum_t = p2.tile([128, VW], DT_A, tag="vsum", name="vsum")
                        nc.sync.dma_start(vsum_t[:], kv_sum_v[t * 128:(t + 1) * 128, :])
                        vp = p1.tile([128, VW + 16], DT_A, tag=f"vpart{t}", name=f"vpart{t}")
                        nc.vector.memset(vp[:, VW:VW + 16], 0.0)
                        nc.vector.tensor_tensor(vp[:, 0:VW], vsum_t[:],
                                                vloc[t][:, 0:VW], OP.subtract)
                        vpart.append(vp)

                    # ---- partner phase: ALiBi rides in the aug rows ----
                    sums_g = p1.tile([16, T], DT_A, tag="sums_g", name="sums_g")
                    o_pad = []
                    for j in range(8):
                        psav = psmm.tile([128, T], F32, tag="mm", name="mm")
                        nc.vector.memset(psav[:], 0.0)
                        attn_phase(j, kpart, qT, vpart, False, psav)
                        oj = o_diag[j]
                        nc.vector.tensor_tensor(oj[:], oj[:], psav[:], OP.add)
                        # softmax denominators sit at rows 48 / 112 (v ones-aug)
                        nc.sync.dma_start(sums_g[2 * j:2 * j + 1, :], oj[48:49, :])
                        nc.sync.dma_start(sums_g[2 * j + 1:2 * j + 2, :], oj[112:113, :])
                        o_pad.append(oj)

                    rec_f = p1.tile([16, T], F32, tag="rec_f", name="rec_f")
                    nc.vector.reciprocal(rec_f[:], sums_g[:])
                    rec_b = p1.tile([16, T], DT_A, tag="rec_b", name="rec_b")
                    nc.vector.tensor_copy(rec_b[:], rec_f[:])

                    o_scaled = []
                    for j in range(8):
                        ra = p4.tile([1, T], DT_A, tag="rec_row", name="rec_row",
                                     bufs=2)
                        nc.sync.dma_start(ra[:], rec_b[2 * j:2 * j + 1, :])
                        rb = p4.tile([1, T], DT_A, tag="rec_row", name="rec_row",
                                     bufs=2)
                        nc.sync.dma_start(rb[:], rec_b[2 * j + 1:2 * j + 2, :])
                        rep = psmm.tile([128, T], F32, tag="mm", name="mm")
                        nc.tensor.matmul(rep[0:64, :], mask48[:], ra[:],
                                         start=True, stop=True, tile_position=(0, 0))
                        nc.tensor.matmul(rep[64:128, :], mask48[:], rb[:],
                                         start=True, stop=True, tile_position=(0, 64))
                        rep_sb = p2.tile([128, T], DT_A, tag="rep_sb", name="rep_sb")
                        nc.vector.tensor_copy(rep_sb[:], rep[:])
                        osj = p1.tile([128, T], DT_A, tag=f"opad{j}", name=f"opad{j}")
                        nc.vector.tensor_tensor(osj[:], o_pad[j][:], rep_sb[:], OP.mult)
                        o_scaled.append(osj)

                    # repack o to dense feature rows for the dense Wo
                    o_dense = [p1.tile([128, T], DT_A, tag=f"oden{k}", name=f"oden{k}")
                               for k in range(KT)]
                    for h in range(H):
                        src = o_scaled[h // 2]
                        sb = (h % 2) * 64
                        for (g, r0, sr, ln) in head_spans(h):
                            nc.sync.dma_start(o_dense[g][r0:r0 + ln, :],
                                              src[sb + sr:sb + sr + ln, :])

                    # ---- Wo (dense) + residual ----
                    for t in range(TT):
                        psw = psmm.tile([128, 1024], F32, tag="mmp", name="mmp", bufs=3)
                        for (n0, nlen) in ((0, 512), (512, 256)):
                            for k in range(KT):
                                nc.tensor.matmul(psw[:, n0:n0 + nlen],
                                                 o_dense[k][:, t * 128:(t + 1) * 128],
                                                 wo_sb[:, k, n0:n0 + nlen],
                                                 start=(k == 0), stop=(k == KT - 1))
                        nc.vector.tensor_tensor(x[t][:], x[t][:], psw[:, 0:D], OP.add)

                    # ================= SwiGLU FFN =================
                    if use_bgu:
                        bgt = p2.tile([1, FF], DT_H, tag="bg_s", name="bg_s")
                        nc.sync.dma_start(bgt[:], bg_d.ap()[l])
                        but = p2.tile([1, FF], DT_H, tag="bu_s", name="bu_s")
                        nc.sync.dma_start(but[:], bu_d.ap()[l])

                    if FP8_FFN and not use_bgu:
                        # fp8 DoubleRow path: weights pre-scaled x64 host-side,
                        # LN acts quantized to fp8; contraction depth 256/inst.
                        DR = mybir.MatmulPerfMode.DoubleRow
                        h8 = [p1.tile([128, 2, T], FP8, tag=f"h8_{k2}",
                                      name=f"h8_{k2}") for k2 in range(KT // 2)]
                        layernorm_to(x, [h8[d // 2][:, d % 2, :] for d in range(KT)])

                        h38 = []
                        for f in range(FT):
                            wgu = p2.tile([128, KT // 2, 2, 256], FP8, tag="wgu_s",
                                          name="wgu_s")
                            nc.sync.dma_start(
                                wgu[:], wgu_d.ap()[l][:, f * 256:(f + 1) * 256]
                                .rearrange("(o i p) n -> p o i n", p=128, i=2))
                            psg = psmm.tile([128, T], F32, tag="mm", name="mm")
                            for k2 in range(KT // 2):
                                nc.tensor.matmul(psg[:], wgu[:, k2, :, 0:128],
                                                 h8[k2][:], start=(k2 == 0),
                                                 stop=(k2 == KT // 2 - 1),
                                                 perf_mode=DR)
                            gsb = p2.tile([128, T], DT_A, tag="g_sb", name="g_sb")
                            nc.scalar.activation(gsb[:], psg[:], AF.Silu,
                                                 scale=1.0 / SCALE_G)
                            psu = psmm.tile([128, T], F32, tag="mm", name="mm")
                            for k2 in range(KT // 2):
                                nc.tensor.matmul(psu[:], wgu[:, k2, :, 128:256],
                                                 h8[k2][:], start=(k2 == 0),
                                                 stop=(k2 == KT // 2 - 1),
                                                 perf_mode=DR)
                            if f % 2 == 0:
                                h38.append(p1.tile([128, 2, T], FP8,
                                                   tag=f"h38_{f // 2}",
                                                   name=f"h38_{f // 2}"))
                            nc.vector.tensor_tensor(h38[f // 2][:, f % 2, :],
                                                    gsb[:], psu[:], OP.mult)

                        for tpair in ((0, 1), (2, 3)):
                            psd = {}
                            for t in tpair:
                                psd[t] = psmm.tile([128, 1024], F32, tag="mmp",
                                                   name="mmp", bufs=3)
                            for f2 in range(FT // 2):
                                wdt = p3.tile([128, 2, D], FP8, tag="wd_s",
                                              name="wd_s")
                                nc.sync.dma_start(
                                    wdt[:], wd_d.ap()[l][f2 * 256:(f2 + 1) * 256, :]
                                    .rearrange("(i p) n -> p i n", p=128))
                                for t in tpair:
                                    for (n0, nlen) in ((0, 512), (512, 256)):
                                        nc.tensor.matmul(
                                            psd[t][:, n0:n0 + nlen],
                                            h38[f2][:, :, t * 128:(t + 1) * 128],
                                            wdt[:, :, n0:n0 + nlen],
                                            start=(f2 == 0),
                                            stop=(f2 == FT // 2 - 1),
                                            perf_mode=DR, skip_group_check=True)
                            for t in tpair:
                                evd = p2.tile([128, D], F32, tag="evd", name="evd")
                                nc.scalar.mul(evd[:], psd[t][:, 0:D],
                                              1.0 / (SCALE_U * SCALE_D))
                                nc.vector.tensor_tensor(x[t][:], x[t][:], evd[:],
                                                        OP.add)
                    else:
                        h2T = [p1.tile([128, T], DT_H, tag=f"hT{k}", name=f"hT{k}")
                               for k in range(KT)]
                        layernorm_to(x, h2T)

                        h3 = []
                        for f in range(FT):
                            wgu = p2.tile([128, KT, 256], DT_H, tag="wgu_s", name="wgu_s")
                            nc.sync.dma_start(
                                wgu[:], wgu_d.ap()[l][:, f * 256:(f + 1) * 256]
                                .rearrange("(o p) n -> p o n", p=128))
                            psg = psmm.tile([128, T], F32, tag="mm", name="mm")
                            for k in range(KT):
                                nc.tensor.matmul(psg[:], wgu[:, k, 0:128], h2T[k][:],
                                                 start=(k == 0),
                                                 stop=(k == KT - 1 and not use_bgu))
                            if use_bgu:
                                nc.tensor.matmul(psg[:], bgt[0:1, f * 128:(f + 1) * 128],
                                                 ones_row[:], start=False, stop=True)
                            gsb = p2.tile([128, T], DT_A, tag="g_sb", name="g_sb")
                            nc.scalar.activation(gsb[:], psg[:], AF.Silu)
                            psu = psmm.tile([128, T], F32, tag="mm", name="mm")
                            for k in range(KT):
                                nc.tensor.matmul(psu[:], wgu[:, k, 128:256], h2T[k][:],
                                                 start=(k == 0),
                                                 stop=(k == KT - 1 and not use_bgu))
                            if use_bgu:
                                nc.tensor.matmul(psu[:], but[0:1, f * 128:(f + 1) * 128],
                                                 ones_row[:], start=False, stop=True)
                            h3f = p1.tile([128, T], DT_A, tag=f"h3_{f}", name=f"h3_{f}")
                            nc.vector.tensor_tensor(h3f[:], gsb[:], psu[:], OP.mult)
                            h3.append(h3f)

                        # down proj in 2 passes of 2 token tiles (wd streamed twice)
                        for tpair in ((0, 1), (2, 3)):
                            psd = {}
                            for t in tpair:
                                psd[t] = psmm.tile([128, 1024], F32, tag="mmp",
                                                   name="mmp", bufs=3)
                            for f in range(FT):
                                wdt = p3.tile([128, D], DT_A, tag="wd_s", name="wd_s")
                                nc.sync.dma_start(wdt[:],
                                                  wd_d.ap()[l][f * 128:(f + 1) * 128, :])
                                for t in tpair:
                                    for (n0, nlen) in ((0, 512), (512, 256)):
                                        nc.tensor.matmul(psd[t][:, n0:n0 + nlen],
                                                         h3[f][:, t * 128:(t + 1) * 128],
                                                         wdt[:, n0:n0 + nlen],
                                                         start=(f == 0),
                                                         stop=(f == FT - 1),
                                                         skip_group_check=True)
                            for t in tpair:
                                nc.vector.tensor_tensor(x[t][:], x[t][:],
                                                        psd[t][:, 0:D], OP.add)

                # ---- final layernorm of the CLS row (token 0) + affine ----
                finw = p1.tile([1, D], F32, tag="finw", name="finw")
                nc.sync.dma_start(finw[:], finw_d.ap())
                finb = p1.tile([1, D], F32, tag="finb", name="finb")
                nc.sync.dma_start(finb[:], finb_d.ap())

                x0r = x[0][0:1, :]
                fst = p2.tile([1, 12], F32, tag="fbnst", name="fbnst")
                nc.vector.bn_stats(fst[:, 0:6], x0r[:, 0:384])
                nc.vector.bn_stats(fst[:, 6:12], x0r[:, 384:768])
                fag = p2.tile([1, 2], F32, tag="fbnag", name="fbnag")
                nc.vector.bn_aggr(fag[:], fst[:])
                nmean = p2.tile([1, 1], F32, tag="fnmean", name="fnmean")
                nc.scalar.mul(nmean[:], fag[:, 0:1], -1.0)
                xc = p1.tile([1, D], F32, tag="fxc", name="fxc")
                nc.vector.tensor_scalar(xc[:], x0r, nmean[:], None, OP.add)
                stdt = p2.tile([1, 1], F32, tag="fstd", name="fstd")
                nc.scalar.activation(stdt[:], fag[:, 1:2], AF.Sqrt, bias=epst[0:1, :])
                rstd = p2.tile([1, 1], F32, tag="frstd", name="frstd")
                nc.vector.reciprocal(rstd[:], stdt[:])
                yt = p1.tile([1, D], F32, tag="fy", name="fy")
                nc.vector.tensor_scalar(yt[:], xc[:], rstd[:], None, OP.mult)
                nc.vector.tensor_tensor(yt[:], yt[:], finw[:], OP.mult)
                nc.vector.tensor_tensor(yt[:], yt[:], finb[:], OP.add)
                nc.sync.dma_start(y_d.ap(), yt[:])

    nc.compile()
    return nc


def prepare_inputs(cls_tokens, cls_token, log_slopes, Wqkv, Wo, Wg, Wu, Wd,
                   ln1_w, ln1_b, ln2_w, ln2_b, fin_w, fin_b):
    """Fold LN affine params into weights, pad heads, build per-core arrays."""
    f32 = np.float32
    scale = 1.0 / math.sqrt(HD)

    slopes_np = np.exp(np.asarray(log_slopes, np.float64))
    wqk = np.zeros((L, D, 2 * QK_D), f32)
    bqk = np.zeros((L, 1, 2 * QK_D), f32)
    wv = np.zeros((L, D, VW), f32)
    bv = np.zeros((L, 1, VW), f32)
    wo = np.zeros((L, D, D), f32)
    wgu = np.zeros((L, D, 2 * FF), f32)
    bg = np.zeros((L, 1, FF), f32)
    bu = np.zeros((L, 1, FF), f32)
    wd = np.zeros((L, FF, D), f32)

    for l in range(L):
        W1 = (np.asarray(Wqkv[l], np.float64) *
              np.asarray(ln1_w[l], np.float64)[None, :])
        b1 = np.asarray(Wqkv[l], np.float64) @ np.asarray(ln1_b[l], np.float64)
        for h in range(H):
            qs = slice(48 * h, 48 * h + 48)
            wqk[l, :, 48 * h:48 * h + 48] = (W1[qs].T * scale)
            bqk[l, 0, 48 * h:48 * h + 48] = b1[qs] * scale
            ks = slice(D + 48 * h, D + 48 * h + 48)
            wqk[l, :, QK_D + 48 * h:QK_D + 48 * h + 48] = W1[ks].T
            bqk[l, 0, QK_D + 48 * h:QK_D + 48 * h + 48] = b1[ks]
            vs = slice(2 * D + 48 * h, 2 * D + 48 * h + 48)
            wv[l, :, 49 * h:49 * h + 48] = W1[vs].T
            bv[l, 0, 49 * h:49 * h + 48] = b1[vs]
            bv[l, 0, 49 * h + 48] = 1.0
        wo[l] = np.asarray(Wo[l], f32).T
        W2g = (np.asarray(Wg[l], np.float64) *
               np.asarray(ln2_w[l], np.float64)[None, :])
        W2u = (np.asarray(Wu[l], np.float64) *
               np.asarray(ln2_w[l], np.float64)[None, :])
        wgu_l = wgu[l].reshape(D, FT, 2, 128)
        wgu_l[:, :, 0, :] = W2g.T.reshape(D, FT, 128)
        wgu_l[:, :, 1, :] = W2u.T.reshape(D, FT, 128)
        bg[l, 0] = np.asarray(Wg[l], np.float64) @ np.asarray(ln2_b[l], np.float64)
        bu[l, 0] = np.asarray(Wu[l], np.float64) @ np.asarray(ln2_b[l], np.float64)
        wd[l] = np.asarray(Wd[l], f32).T

    use_bqk = bool(np.any(bqk != 0))
    use_bgu = bool(np.any(bg != 0) or np.any(bu != 0))

    sid = np.zeros((128, H * 128), np.float16)
    for h in range(H):
        sid[:, h * 128:(h + 1) * 128] = -slopes_np[h] * np.eye(128)

    # diagonal |i-j| table over the local half (identical for every core)
    kk = np.arange(T, dtype=np.float64)
    dist = np.abs(kk[:, None] - kk[None, :]).astype(np.float16)

    # slope quantized once so both aug rows share the exact same factor
    s64 = np.float16(64.0 * slopes_np).astype(np.float64)

    x_full = np.concatenate(
        [np.broadcast_to(np.asarray(cls_token, f32), (B, 1, D)),
         np.asarray(cls_tokens, f32)], axis=1)  # (B, S, D)

    np_a = mybir.dt.np(DT_A)
    np_h = mybir.dt.np(DT_H)
    if FP8_FFN and not use_bgu:
        np_8 = mybir.dt.np(FP8)
        wgu_sc = wgu.reshape(L, D, FT, 2, 128).copy()
        wgu_sc[:, :, :, 0, :] *= SCALE_G
        wgu_sc[:, :, :, 1, :] *= SCALE_U
        wgu_cast = wgu_sc.reshape(L, D, 2 * FF).astype(np_8)
        wd_cast = (wd * SCALE_D).astype(np_8)
    else:
        wgu_cast = wgu.astype(np_h)
        wd_cast = wd.astype(np_a)
    common = {
        "wqk": wqk.astype(np_h), "wv": wv.astype(np_h), "bv": bv.astype(np_h),
        "wo": wo.astype(np_a), "wgu": wgu_cast,
        "wd": wd_cast,
        "sid": sid, "dist": dist,
        "finw": np.asarray(fin_w, f32).reshape(1, D),
        "finb": np.asarray(fin_b, f32).reshape(1, D),
    }
    if use_bqk:
        common["bqk"] = bqk.astype(np_h)
    if use_bgu:
        common["bg"] = bg.astype(np_h)
        common["bu"] = bu.astype(np_h)

    in_maps = []
    for c in range(NCORES):
        b, half = c // 2, c % 2
        q0 = T * half
        sgn = 1.0 if half == 1 else -1.0
        kpos = (1 - half) * T + np.arange(T, dtype=np.float64)  # partner global
        qpos = q0 + np.arange(T, dtype=np.float64)
        kaug = np.zeros((2 * H, T), np.float16)
        qaug = np.zeros((2 * H, T), np.float16)
        for h in range(H):
            kaug[2 * h] = np.float16(-sgn * s64[h])
            kaug[2 * h + 1] = np.float16(sgn * (kpos - 512.0) / 64.0)
            qaug[2 * h] = np.float16((qpos - 512.0) / 64.0)
            qaug[2 * h + 1] = np.float16(s64[h])
        m = dict(common)
        m["x0"] = np.ascontiguousarray(x_full[b, q0:q0 + T])
        m["kaug"] = kaug
        m["qaug"] = qaug
        in_maps.append(m)
    return in_maps, use_bqk, use_bgu


def kernel(**inputs):
    in_maps, use_bqk, use_bgu = prepare_inputs(**inputs)
    slopes = np.exp(np.asarray(inputs["log_slopes"], np.float64))
    key = (use_bqk, use_bgu, tuple(np.round(slopes, 10)))
    if key not in _NC_CACHE:
        _NC_CACHE[key] = build_nc(use_bqk, use_bgu, slopes=slopes)
    nc = _NC_CACHE[key]
    res = run_bass_kernel_spmd(nc, in_maps, core_ids=list(range(NCORES)))
    out = np.stack([res.results[2 * b]["y"][0] for b in range(B)])
    return out.astype(np.float32)
